# revision 27
# baseline (speedup 1.0000x reference)
"""Trainium2 Bass kernel for a 2-layer GAT + global max pool + linear head.

Contract: kernel(**inputs) takes FULL unsharded inputs (as produced by
reference.setup_inputs) and returns the FULL [N_GRAPHS, N_CLASSES] float32
output.

Design (per-destination-row layout):
- Nodes are degree-sorted within each core block so each 128-node dst tile
  has near-uniform in-degree; each dst node owns one SBUF partition row and
  its (padded) incident edges sit along the free dim.  The segment softmax
  becomes plain free-dim reductions -- no masks, no per-edge matmuls.
- Edge payloads are fetched with batched dma_gather (int16 indices, <=1024
  per call, spread over 4 SWDGE queues).  The 50176-row node table exceeds
  the int16 range, so edges gather from two overlapping 32768-row windows
  ([0, 32768) and [17408, 50176)); edges with sources in the overlap are
  assigned to whichever window balances the per-row split.  Both windows'
  chunks land side by side in one SBUF tile so per-edge math is single-pass.
- Node tables are built per-core and AllGathered in 7 chunks interleaved
  with compute; the global table layout is chunk-major ([chunk][core][row])
  so each chunk's collective output is contiguous.
- Padded edge slots point at designated pad rows (one per window) whose
  a_src logit is -1000, so exp(leaky_relu(...)) underflows to exactly 0.
- Layer-2 outputs are scattered back to original node order with
  dma_scatter_add, then max-pooled per graph with precomputed segment masks
  and AllReduced (max) across cores.

Self-contained: hardcodes all shapes; reads nothing from /root/problem.
"""
import sys

sys.path.insert(0, "/opt/trn_rl_repo")
sys.path.insert(0, "/opt/pypackages")

import numpy as np

# ---------------------------------------------------------------- constants
N = 50000
IN_CH = 128
HID = 32
OUT_CH = 64
HEADS = 4
NCLS = 32
NG = 64
NC = 8
NEG = 0.2
P = 128
NLOC = N // NC              # 6250
TILES = -(-NLOC // P)       # 49
TPAD = TILES * P            # 6272
VPAD = NC * TPAD            # 50176
WIN = 32768
OFFB = VPAD - WIN           # 17408
CL1 = HEADS * HID           # 128
CL2 = HEADS * OUT_CH        # 256
R1 = 256                    # table1 row (halfs): h1(128) | as1(4) | pad
R2 = 384                    # table2 row (halfs): h2(256) | as2(4) | pad
POOLP = OUT_CH
OFF = 1024.0
NCH = 7                     # table chunks (collective granularity)
TPG = TILES // NCH          # 7 tiles per chunk
CHR = TPG * P               # 896 local rows per chunk
GCH = NC * CHR              # 7168 global rows per chunk
NPADF = 11                  # pad slots at the front of each core's order
MAXW = 8                    # dma_gather caps out near 1024 indices/call
NQ = 4                      # SWDGE queues

# chunk-major global row of (core k, local sorted row r)
def _gmap(k, r):
    return (r // CHR) * GCH + k * CHR + (r % CHR)


PAD_A = _gmap(0, 0)                 # = 0, in window A (front pad slot)
PAD_B = _gmap(0, TPAD - 1)          # = 43903, in window B (back pad slot)
assert PAD_A < WIN and PAD_B >= OFFB


# ---------------------------------------------------------------- host prep
def _sort_and_split(deg, src, dst):
    """Per-core degree sort (pads split front/back) + window assignment.

    Returns sp_cm[node] = core*TPAD + slot (core-major), g[node] = chunk-major
    global table row, per-node nA, and shared per-tile (WA, WB)."""
    def make_sorted_pos(keys):
        sp = np.empty(N, np.int64)
        for k in range(NC):
            lo = k * NLOC
            order = np.lexsort(tuple(kk[lo:lo + NLOC] for kk in reversed(keys)))
            sp[lo + order] = k * TPAD + NPADF + np.arange(NLOC)
        return sp

    def to_g(sp):
        k = sp // TPAD
        r = sp % TPAD
        return (r // CHR) * GCH + k * CHR + (r % CHR)

    def forced_counts(g_):
        srow = g_[src]
        nAf = np.bincount(dst, weights=(srow < OFFB).astype(np.float64),
                          minlength=N).astype(np.int64)
        nBf = np.bincount(dst, weights=(srow >= WIN).astype(np.float64),
                          minlength=N).astype(np.int64)
        return nAf, nBf

    sp = make_sorted_pos([-deg])
    nAf, nBf = forced_counts(to_g(sp))
    f = deg - nAf - nBf
    nA_prelim = nAf + np.clip((deg + 1) // 2 - nAf, 0, f)
    sp = make_sorted_pos([-deg, nA_prelim])
    g = to_g(sp)
    nAf, nBf = forced_counts(g)
    f = deg - nAf - nBf

    nAf_s = np.zeros(VPAD, np.int64)
    f_s = np.zeros(VPAD, np.int64)
    d_s = np.zeros(VPAD, np.int64)
    nAf_s[sp] = nAf
    f_s[sp] = f
    d_s[sp] = deg
    nAf_t = nAf_s.reshape(NC, TILES, P)
    f_t = f_s.reshape(NC, TILES, P)
    d_t = d_s.reshape(NC, TILES, P)
    WA = np.zeros(TILES, np.int64)
    WB = np.zeros(TILES, np.int64)
    nA_slot = np.zeros(VPAD, np.int64).reshape(NC, TILES, P)
    for t in range(TILES):
        best = None
        for tau in range(0, int(d_t[:, t].max()) + 2):
            nA = np.clip(tau, nAf_t[:, t], nAf_t[:, t] + f_t[:, t])
            wa = int(nA.max())
            wb = int((d_t[:, t] - nA).max())
            if best is None or wa + wb < best[0]:
                best = (wa + wb, wa, wb, tau)
        WA[t], WB[t] = best[1], best[2]
        nA_slot[:, t] = np.clip(best[3], nAf_t[:, t], nAf_t[:, t] + f_t[:, t])
    WA = np.maximum(WA, 1)
    WB = np.maximum(WB, 1)
    return sp, g, nA_slot.reshape(-1), [int(w) for w in WA], [int(w) for w in WB]


def _wrap16(arr, ni):
    """Index vector [ni] -> dma_gather idx layout [128, ni//16] int16."""
    a = arr.reshape(ni // 16, 16).T.astype(np.int16)
    return np.ascontiguousarray(np.tile(a, (8, 1)))


def _chunks(w):
    """Split a width into near-equal chunks of at most MAXW columns."""
    n = -(-w // MAXW)
    base = w // n
    return [base + (1 if i < w % n else 0) for i in range(n)]


def _build_eidx(sp, g, nA_slot, WAs, WBs, src, dst):
    """Per-core packed gather indices for both windows, all tiles, chunked."""
    srow = g[src]
    drow = sp[dst]
    eorder = np.argsort(drow, kind="stable")
    srow_s = srow[eorder]
    drow_s = drow[eorder]
    bounds = np.searchsorted(drow_s, np.arange(VPAD + 1))

    eidxA = [[] for _ in range(NC)]
    eidxB = [[] for _ in range(NC)]
    for k in range(NC):
        for t in range(TILES):
            WA, WB = WAs[t], WBs[t]
            arrA = np.full((WA, P), PAD_A, np.int64)
            arrB = np.full((WB, P), PAD_B - OFFB, np.int64)
            for p in range(P):
                gslot = k * TPAD + t * P + p
                a, b = bounds[gslot], bounds[gslot + 1]
                s = srow_s[a:b]
                inA = s < OFFB
                inB = s >= WIN
                flex = ~(inA | inB)
                nA = int(nA_slot[gslot])
                aa = nA - int(inA.sum())
                fi = np.flatnonzero(flex)
                listA = np.concatenate([s[inA], s[fi[:aa]]])
                listB = np.concatenate([s[inB], s[fi[aa:]]])
                arrA[:len(listA), p] = listA
                arrB[:len(listB), p] = listB - OFFB
            c0 = 0
            for w in _chunks(WA):
                eidxA[k].append(_wrap16(arrA[c0:c0 + w].reshape(-1), P * w))
                c0 += w
            c0 = 0
            for w in _chunks(WB):
                eidxB[k].append(_wrap16(arrB[c0:c0 + w].reshape(-1), P * w))
                c0 += w
    eidxA = [np.concatenate(e, axis=1) for e in eidxA]
    eidxB = [np.concatenate(e, axis=1) for e in eidxB]
    return eidxA, eidxB


def _build_pool(batch):
    """Per-core segment masks / graph-slot matrix on ORIGINAL node order."""
    runs = [[[] for _ in range(TILES)] for _ in range(NC)]
    SEG = 1
    for k in range(NC):
        bl = batch[k * NLOC:(k + 1) * NLOC].astype(np.int64)
        for t in range(TILES):
            lo = t * P
            hi = min((t + 1) * P, NLOC)
            if hi <= lo:
                continue
            seg = bl[lo:hi]
            cuts = np.flatnonzero(np.diff(seg)) + 1
            edges = np.concatenate([[0], cuts, [len(seg)]])
            for i in range(len(edges) - 1):
                runs[k][t].append((int(seg[edges[i]]), int(edges[i]), int(edges[i + 1])))
            SEG = max(SEG, len(edges) - 1)
    STOT = TILES * SEG
    segmask = np.zeros((NC, TILES, POOLP, SEG * P), np.float16)
    M = np.zeros((NC, POOLP, NG * STOT), np.float16)
    for k in range(NC):
        for t in range(TILES):
            for s, (gid, lo, hi) in enumerate(runs[k][t]):
                segmask[k, t, :, s * P + lo:s * P + hi] = 1.0
                M[k, :, gid * STOT + t * SEG + s] = 1.0
    return SEG, STOT, segmask, M


def _preprocess(x, edge_index, batch):
    src = np.concatenate([edge_index[0].astype(np.int64), np.arange(N, dtype=np.int64)])
    dst = np.concatenate([edge_index[1].astype(np.int64), np.arange(N, dtype=np.int64)])
    deg = np.bincount(dst, minlength=N)
    sp, g, nA_slot, WAs, WBs = _sort_and_split(deg, src, dst)
    eidxA, eidxB = _build_eidx(sp, g, nA_slot, WAs, WBs, src, dst)
    SEG, STOT, segmask, M = _build_pool(batch)

    # per-core sorted x, transposed fp16
    xT16 = []
    for k in range(NC):
        xs = np.zeros((TPAD, IN_CH), np.float32)
        loc = sp[k * NLOC:(k + 1) * NLOC] - k * TPAD
        xs[loc] = x[k * NLOC:(k + 1) * NLOC]
        xT16.append(np.ascontiguousarray(xs.T.astype(np.float16)))

    # scatter indices: sorted slot (p, tile c) -> original local row,
    # grouped into NCH scatter groups of TPG tiles
    so2 = []
    for k in range(NC):
        orig_of_slot = np.arange(TPAD, dtype=np.int64)  # dump rows for pads
        loc = sp[k * NLOC:(k + 1) * NLOC] - k * TPAD
        orig_of_slot[loc] = np.arange(NLOC)
        per_tile = orig_of_slot.reshape(TILES, P)
        grp = []
        for gg in range(NCH):
            seg = per_tile[gg * TPG:(gg + 1) * TPG]
            grp.append(_wrap16(seg.reshape(-1), TPG * P))
        so2.append(np.concatenate(grp, axis=1))

    meta = dict(WAs=WAs, WBs=WBs, SEG=SEG, STOT=STOT,
                CWA=8 * sum(WAs), CWB=8 * sum(WBs))
    per_core = [dict(eidxA=eidxA[k], eidxB=eidxB[k], so2=so2[k], xT16=xT16[k],
                     segmask=segmask[k].reshape(TILES * POOLP, SEG * P),
                     Mfull=M[k]) for k in range(NC)]
    return meta, per_core, sp


def _weights_host(sp, x, W1, a_s1, a_d1, b1, W2, a_s2, a_d2, b2, Wl, bl):
    f16, f32 = np.float16, np.float32
    B1as = np.zeros((IN_CH, HEADS), f32)
    B1ad = np.zeros((IN_CH, HEADS), f32)
    for h in range(HEADS):
        B1as[:, h] = W1[h * HID:(h + 1) * HID, :].T @ a_s1[h]
        B1ad[:, h] = W1[h * HID:(h + 1) * HID, :].T @ a_d1[h]
    B2as = np.zeros((CL1, HEADS), f32)
    B2ad = np.zeros((CL1, HEADS), f32)
    for h in range(HEADS):
        B2as[:, h] = W2[h * OUT_CH:(h + 1) * OUT_CH, :].T @ a_s2[h]
        B2ad[:, h] = W2[h * OUT_CH:(h + 1) * OUT_CH, :].T @ a_d2[h]

    WB1 = np.concatenate([W1.T, B1as], axis=1).astype(f16)          # [128, 132]
    WB2 = np.concatenate([W2.T, B2as, B2ad], axis=1).astype(f16)    # [128, 264]

    # per-core a_dst1 logits of local sorted nodes (computed on host)
    alS1 = []
    for k in range(NC):
        xs = np.zeros((TPAD, IN_CH), np.float32)
        loc = sp[k * NLOC:(k + 1) * NLOC] - k * TPAD
        xs[loc] = x[k * NLOC:(k + 1) * NLOC]
        ad1 = (xs @ B1ad).astype(f16).reshape(TILES, P, HEADS)
        alS1.append(np.ascontiguousarray(
            ad1.transpose(1, 0, 2).reshape(P, TILES * HEADS)))

    shared = dict(
        WB1=WB1, WB2=WB2,
        b1row=np.tile(b1.astype(f32)[None, :], (P, 1)),
        b2row=np.tile(b2.astype(f32)[None, :], (P, 1)),
        blrow=np.tile(bl.astype(f32)[None, :], (NG, 1)),
        WlT32=Wl.T.astype(f32).copy(),
        m1000=np.full((1, 4), -1000.0, f16),
    )
    return shared, alS1


# ---------------------------------------------------------------- program
def _build(meta):
    import concourse.bass as bass
    import concourse.bacc as bacc
    import concourse.tile as tile
    from concourse import mybir

    f16, f32, i16 = mybir.dt.float16, mybir.dt.float32, mybir.dt.int16
    Alu = mybir.AluOpType
    Act = mybir.ActivationFunctionType
    from concourse.masks import make_identity

    WAs, WBs = meta["WAs"], meta["WBs"]
    SEG, STOT = meta["SEG"], meta["STOT"]
    CWA, CWB = meta["CWA"], meta["CWB"]

    nc = bacc.Bacc("TRN2", target_bir_lowering=False, debug=False,
                   num_devices=NC, num_swdge_queues=NQ)

    pr = {}
    def param(name, shape, dt, out=False):
        pr[name] = nc.declare_dram_parameter(name, list(shape), dt, isOutput=out)
        return pr[name]

    param("xT16", [P, TPAD], f16)
    param("eidxA", [P, CWA], i16)
    param("eidxB", [P, CWB], i16)
    param("so2", [P, NCH * (TPG * P // 16)], i16)
    param("alS1", [P, TILES * HEADS], f16)
    param("segmask", [TILES * POOLP, SEG * P], f16)
    param("Mfull", [POOLP, NG * STOT], f16)
    param("WB1", [P, 132], f16)
    param("WB2", [P, 264], f16)
    param("b1row", [P, CL1], f32)
    param("b2row", [P, OUT_CH], f32)
    param("blrow", [NG, NCLS], f32)
    param("WlT32", [OUT_CH, NCLS], f32)
    param("m1000", [1, 4], f16)
    out_ext = param("out_logits", [NG, NCLS], f32, out=True)

    rg = [list(range(NC))]
    qctr = [0]
    def next_q():
        q = qctr[0] % NQ
        qctr[0] += 1
        return q

    with tile.TileContext(nc) as tc:
        with tc.tile_pool(name="const", bufs=1) as cp, \
             tc.tile_pool(name="dram", bufs=1, space="DRAM") as dp:

            def ld(name, shape, dt):
                t_ = cp.tile(list(shape), dt, name="c_" + name)
                nc.sync.dma_start(t_[:], pr[name][:])
                return t_

            WB1s = ld("WB1", [P, 132], f16)
            WB2s = ld("WB2", [P, 264], f16)
            b1row = ld("b1row", [P, CL1], f32)
            b2row = ld("b2row", [P, OUT_CH], f32)
            blrow = ld("blrow", [NG, NCLS], f32)
            WlT32 = ld("WlT32", [OUT_CH, NCLS], f32)
            m1000t = ld("m1000", [1, 4], f16)
            Mfull = ld("Mfull", [POOLP, NG * STOT], f16)
            idxA = ld("eidxA", [P, CWA], i16)
            idxB = ld("eidxB", [P, CWB], i16)
            so2t = ld("so2", [P, NCH * (TPG * P // 16)], i16)
            alS1 = ld("alS1", [P, TILES * HEADS], f16)

            ident = cp.tile([P, P], f32)
            make_identity(nc, ident[:])
            alS2 = cp.tile([P, TILES * HEADS], f16)
            o2acc = cp.tile([P, TILES * OUT_CH], f32)
            partial = cp.tile([POOLP, STOT], f32)

            tb1_loc = dp.tile([TPAD, R1], f16)
            tb1_full = dp.tile([VPAD, R1], f16)
            tb2_loc = dp.tile([TPAD, R2], f16)
            tb2_full = dp.tile([VPAD, R2], f16)
            o2_d = dp.tile([TPAD, OUT_CH], f32)
            ar_in = dp.tile([POOLP, NG], f32)
            ar_out = dp.tile([POOLP, NG], f32)

            # ============ phase T1: local table1 + chunked AllGather
            with tc.tile_pool(name="t1", bufs=6) as p1, \
                 tc.tile_pool(name="t1ps", bufs=4, space="PSUM") as p1ps:
                for t in range(TILES):
                    xsl = p1.tile([P, P], f16, tag="xsl")
                    nc.sync.dma_start(xsl[:], pr["xT16"][:, t * P:(t + 1) * P])
                    ps = p1ps.tile([P, 132], f32, tag="ps")
                    nc.tensor.matmul(ps[:], lhsT=xsl[:], rhs=WB1s[:],
                                     start=True, stop=True)
                    h16 = p1.tile([P, 132], f16, tag="h16")
                    nc.scalar.activation(h16[:], ps[:], Act.Copy)
                    nc.sync.dma_start(tb1_loc[t * P:(t + 1) * P, 0:132], h16[:])
                    if t == 0:
                        nc.sync.dma_start(tb1_loc[0:1, 128:132], m1000t[:])
                    if t == TILES - 1:
                        nc.sync.dma_start(tb1_loc[TPAD - 1:TPAD, 128:132],
                                          m1000t[:])
                    if t % TPG == TPG - 1:
                        c = t // TPG
                        nc.gpsimd.collective_compute(
                            "AllGather", mybir.AluOpType.bypass,
                            replica_groups=rg,
                            ins=[tb1_loc[c * CHR:(c + 1) * CHR, :].opt()],
                            outs=[tb1_full[c * GCH:(c + 1) * GCH, :].opt()])
            tc.strict_bb_all_engine_barrier()

            # ============ edge layer (shared for L1/L2)
            def edge_layer(t, pe, CL, ROW, tabw, colA, colB, adS, out_cb):
                WA, WB = WAs[t], WBs[t]
                W = WA + WB
                CH = CL // HEADS
                hg = pe.tile([P, W * ROW], f16, tag="hg")
                co = 0
                ca = colA
                for w in _chunks(WA):
                    nc.gpsimd.dma_gather(
                        out_ap=hg[:, co * ROW:(co + w) * ROW]
                            .rearrange("p (w r) -> p w r", w=w),
                        in_ap=tabw[0][:], idxs_ap=idxA[:, ca:ca + 8 * w],
                        num_idxs=P * w, num_idxs_reg=P * w, elem_size=ROW,
                        queue_num=next_q())
                    co += w
                    ca += 8 * w
                cb = colB
                for w in _chunks(WB):
                    nc.gpsimd.dma_gather(
                        out_ap=hg[:, co * ROW:(co + w) * ROW]
                            .rearrange("p (w r) -> p w r", w=w),
                        in_ap=tabw[1][:], idxs_ap=idxB[:, cb:cb + 8 * w],
                        num_idxs=P * w, num_idxs_reg=P * w, elem_size=ROW,
                        queue_num=next_q())
                    co += w
                    cb += 8 * w
                hg3 = hg[:].rearrange("p (w r) -> p w r", w=W)
                ad = adS[:, HEADS * t:HEADS * (t + 1)]
                sc = pe.tile([P, HEADS * W], f32, tag="sc")
                nc.vector.tensor_tensor(
                    out=sc[:].rearrange("p (h w) -> p h w", h=HEADS),
                    in0=hg3[:, :, CL:CL + HEADS].rearrange("p w h -> p h w"),
                    in1=ad[:, :, None].to_broadcast([P, HEADS, W]),
                    op=Alu.add)
                lr = pe.tile([P, HEADS * W], f32, tag="lr")
                nc.vector.scalar_tensor_tensor(out=lr[:], in0=sc[:], scalar=NEG,
                                               in1=sc[:], op0=Alu.mult, op1=Alu.max)
                ex = pe.tile([P, HEADS * W], f16, tag="ex")
                den = pe.tile([P, HEADS], f32, tag="den")
                for h in range(HEADS):
                    nc.scalar.activation(ex[:, h * W:(h + 1) * W],
                                         lr[:, h * W:(h + 1) * W], Act.Exp,
                                         accum_out=den[:, h:h + 1])
                dei = pe.tile([P, HEADS], f32, tag="dei")
                nc.vector.tensor_scalar_add(out=dei[:], in0=den[:], scalar1=1e-16)
                rden = pe.tile([P, HEADS], f32, tag="rden")
                nc.vector.reciprocal(rden[:], dei[:])
                w1 = pe.tile([P, CL * W], f16, tag="w1")
                nc.vector.tensor_tensor(
                    out=w1[:].rearrange("p (h c w) -> p h c w", h=HEADS, c=CH),
                    in0=hg3[:, :, 0:CL].rearrange("p w (h c) -> p h c w", h=HEADS),
                    in1=ex[:].rearrange("p (h w) -> p h w", h=HEADS)[:, :, None, :]
                        .to_broadcast([P, HEADS, CH, W]),
                    op=Alu.mult)
                nums = pe.tile([P, CL], f32, tag="nums")
                nc.vector.reduce_sum(nums[:],
                                     w1[:].rearrange("p (c w) -> p c w", c=CL),
                                     axis=mybir.AxisListType.X)
                num = pe.tile([P, CL], f32, tag="num")
                nc.vector.tensor_tensor(
                    out=num[:].rearrange("p (h c) -> p h c", h=HEADS),
                    in0=nums[:].rearrange("p (h c) -> p h c", h=HEADS),
                    in1=rden[:, :, None].to_broadcast([P, HEADS, CH]),
                    op=Alu.mult)
                out_cb(t, num, pe)

            # ============ layer 1 (epilogue builds table2, chunked AllGather)
            with tc.tile_pool(name="l1", bufs=4) as pe1, \
                 tc.tile_pool(name="l1ps", bufs=3, space="PSUM") as pe1ps:
                colA = colB = 0
                def epi1(t, num, pe):
                    u32 = pe.tile([P, CL1], f32, tag="u32")
                    nc.vector.tensor_tensor(out=u32[:], in0=num[:], in1=b1row[:],
                                            op=Alu.add)
                    ur = pe.tile([P, CL1], f32, tag="ur")
                    nc.vector.tensor_scalar(out=ur[:], in0=u32[:], scalar1=0.0,
                                            scalar2=None, op0=Alu.max)
                    uTps = pe1ps.tile([P, P], f32, tag="uTps")
                    nc.tensor.transpose(out=uTps[:], in_=ur[:], identity=ident[:])
                    uT16 = pe.tile([P, P], f16, tag="uT16")
                    nc.scalar.activation(uT16[:], uTps[:], Act.Copy)
                    ps2 = pe1ps.tile([P, 264], f32, tag="ps2")
                    nc.tensor.matmul(ps2[:], lhsT=uT16[:], rhs=WB2s[:],
                                     start=True, stop=True)
                    h216 = pe.tile([P, 264], f16, tag="h216")
                    nc.scalar.activation(h216[:], ps2[:], Act.Copy)
                    nc.sync.dma_start(tb2_loc[t * P:(t + 1) * P, 0:260],
                                      h216[:, 0:260])
                    nc.vector.tensor_copy(out=alS2[:, HEADS * t:HEADS * (t + 1)],
                                          in_=h216[:, 260:264])
                    if t == 0:
                        nc.sync.dma_start(tb2_loc[0:1, 256:260], m1000t[:])
                    if t == TILES - 1:
                        nc.sync.dma_start(tb2_loc[TPAD - 1:TPAD, 256:260],
                                          m1000t[:])

                for t in range(TILES):
                    edge_layer(t, pe1, CL1, R1,
                               (tb1_full[0:WIN, :], tb1_full[OFFB:VPAD, :]),
                               colA, colB, alS1, epi1)
                    colA += 8 * WAs[t]
                    colB += 8 * WBs[t]

            for c in range(NCH):
                nc.gpsimd.collective_compute(
                    "AllGather", mybir.AluOpType.bypass, replica_groups=rg,
                    ins=[tb2_loc[c * CHR:(c + 1) * CHR, :].opt()],
                    outs=[tb2_full[c * GCH:(c + 1) * GCH, :].opt()])
            # chunked-slice collectives are not reliably dependency-tracked
            # against the custom-DMA gathers that read their outputs; pin
            # ordering explicitly
            tc.strict_bb_all_engine_barrier()

            # ============ layer 2 (big-W head tiles at bufs=2, tail at bufs=3)
            def epi2(t, num, pe):
                o2p = pe.tile([P, OUT_CH], f32, tag="o2p")
                nc.vector.reduce_sum(
                    o2p[:],
                    num[:].rearrange("p (h c) -> p c h", h=HEADS),
                    axis=mybir.AxisListType.X)
                nc.vector.scalar_tensor_tensor(
                    out=o2acc[:, OUT_CH * t:OUT_CH * (t + 1)],
                    in0=o2p[:], scalar=1.0 / HEADS, in1=b2row[:],
                    op0=Alu.mult, op1=Alu.add)

            TSPLIT = next((t for t in range(TILES) if WAs[t] + WBs[t] <= 28),
                          TILES)
            colA = colB = 0
            with tc.tile_pool(name="l2z", bufs=1) as pz:
                zt = pz.tile([P, TILES * OUT_CH], f32, tag="zt")
                nc.vector.memset(zt[:], 0.0)
                nc.sync.dma_start(
                    o2_d[:].rearrange("(a b) r -> a (b r)", b=TILES), zt[:])
            with tc.tile_pool(name="l2a", bufs=2) as pe2a:
                for t in range(TSPLIT):
                    edge_layer(t, pe2a, CL2, R2,
                               (tb2_full[0:WIN, :], tb2_full[OFFB:VPAD, :]),
                               colA, colB, alS2, epi2)
                    colA += 8 * WAs[t]
                    colB += 8 * WBs[t]
            with tc.tile_pool(name="l2b", bufs=3) as pe2b:
                for t in range(TSPLIT, TILES):
                    edge_layer(t, pe2b, CL2, R2,
                               (tb2_full[0:WIN, :], tb2_full[OFFB:VPAD, :]),
                               colA, colB, alS2, epi2)
                    colA += 8 * WAs[t]
                    colB += 8 * WBs[t]

            # ============ scatter to original order + pooled max + head
            with tc.tile_pool(name="fin", bufs=3) as fp, \
                 tc.tile_pool(name="finps", bufs=3, space="PSUM") as fps:
                tc.strict_bb_all_engine_barrier()
                GW = TPG * P // 16
                for gg in range(NCH):
                    nc.gpsimd.dma_scatter_add(
                        out_ap=o2_d[:],
                        in_ap=o2acc[:, gg * TPG * OUT_CH:(gg + 1) * TPG * OUT_CH]
                            .rearrange("p (t r) -> p t r", t=TPG),
                        idxs_ap=so2t[:, gg * GW:(gg + 1) * GW],
                        num_idxs=TPG * P, num_idxs_reg=TPG * P,
                        elem_size=OUT_CH, queue_num=next_q())
                tc.strict_bb_all_engine_barrier()

                for t in range(TILES):
                    o2l = fp.tile([P, OUT_CH], f32, tag="o2l")
                    nc.sync.dma_start(o2l[:], o2_d[t * P:(t + 1) * P, :])
                    o2Tps = fps.tile([POOLP, P], f32, tag="o2Tps")
                    nc.tensor.transpose(out=o2Tps[:], in_=o2l[:], identity=ident[:])
                    o2T = fp.tile([POOLP, P], f32, tag="o2T")
                    nc.scalar.activation(o2T[:], o2Tps[:], Act.Copy)
                    sm = fp.tile([POOLP, SEG * P], f16, tag="sm")
                    nc.sync.dma_start(sm[:], pr["segmask"][t * POOLP:(t + 1) * POOLP, :])
                    tmp = fp.tile([POOLP, SEG * P], f32, tag="ptmp")
                    nc.vector.scalar_tensor_tensor(
                        out=tmp[:].rearrange("q (s d) -> q s d", s=SEG),
                        in0=o2T[:][:, None, :].to_broadcast([POOLP, SEG, P]),
                        scalar=OFF, in1=sm[:].rearrange("q (s d) -> q s d", s=SEG),
                        op0=Alu.add, op1=Alu.mult)
                    nc.vector.reduce_max(
                        partial[:, t * SEG:(t + 1) * SEG],
                        tmp[:].rearrange("q (s d) -> q s d", s=SEG),
                        axis=mybir.AxisListType.X)

                tmpg = fp.tile([POOLP, NG * STOT], f32, tag="tmpg")
                nc.vector.scalar_tensor_tensor(
                    out=tmpg[:], in0=partial[:][:, None, :].to_broadcast([POOLP, NG, STOT]),
                    scalar=0.0, in1=Mfull[:].rearrange("q (g s) -> q g s", g=NG),
                    op0=Alu.add, op1=Alu.mult)
                pooled = fp.tile([POOLP, NG], f32, tag="pooled")
                nc.vector.reduce_max(pooled[:],
                                     tmpg[:].rearrange("q (g s) -> q g s", g=NG),
                                     axis=mybir.AxisListType.X)
                pooled2 = fp.tile([POOLP, NG], f32, tag="pooled2")
                nc.vector.tensor_scalar_add(out=pooled2[:], in0=pooled[:],
                                            scalar1=-OFF)
                nc.sync.dma_start(ar_in[:], pooled2[:])
                nc.gpsimd.collective_compute(
                    "AllReduce", mybir.AluOpType.max, replica_groups=rg,
                    ins=[ar_in[:].opt()], outs=[ar_out[:].opt()])
                pooledF = fp.tile([POOLP, NG], f32, tag="pooledF")
                nc.sync.dma_start(pooledF[:], ar_out[:])
                lps = fps.tile([NG, NCLS], f32, tag="lps")
                nc.tensor.matmul(lps[:], lhsT=pooledF[:], rhs=WlT32[:],
                                 start=True, stop=True)
                logits = fp.tile([NG, NCLS], f32, tag="logits")
                nc.vector.tensor_tensor(out=logits[:], in0=lps[:], in1=blrow[:],
                                        op=Alu.add)
                nc.sync.dma_start(out_ext[:], logits[:])

    nc.compile()
    return nc


# ---------------------------------------------------------------- runner
def _kernel_impl(inputs, trace=False, trace_kwargs=None):
    x = np.asarray(inputs["x"], np.float32)
    edge_index = np.asarray(inputs["edge_index"]).astype(np.int64)
    batch = np.asarray(inputs["batch"]).astype(np.int64)
    meta, per_core, sp = _preprocess(x, edge_index, batch)
    shared, alS1 = _weights_host(
        sp, x,
        np.asarray(inputs["W1"], np.float32), np.asarray(inputs["a_src1"], np.float32),
        np.asarray(inputs["a_dst1"], np.float32), np.asarray(inputs["b1"], np.float32),
        np.asarray(inputs["W2"], np.float32), np.asarray(inputs["a_src2"], np.float32),
        np.asarray(inputs["a_dst2"], np.float32), np.asarray(inputs["b2"], np.float32),
        np.asarray(inputs["Wl"], np.float32), np.asarray(inputs["bl"], np.float32))
    nc = _build(meta)
    in_maps = []
    for k in range(NC):
        m = dict(shared)
        m.update(per_core[k])
        m["alS1"] = alS1[k]
        in_maps.append({kk: np.ascontiguousarray(v) for kk, v in m.items()})
    from concourse.bass_utils import run_bass_kernel_spmd
    res = run_bass_kernel_spmd(nc, in_maps, core_ids=list(range(NC)),
                               trace=trace, **(trace_kwargs or {}))
    return np.asarray(res.results[0]["out_logits"], np.float32), res


def kernel(**inputs):
    return _kernel_impl(inputs)[0]


# revision 32
# speedup vs baseline: 1.0690x; 1.0690x over previous
"""Trainium2 Bass kernel for a 2-layer GAT + global max pool + linear head.

Contract: kernel(**inputs) takes FULL unsharded inputs (as produced by
reference.setup_inputs) and returns the FULL [N_GRAPHS, N_CLASSES] float32
output.

Design (per-destination-row layout):
- Nodes are degree-sorted within each core block so each 128-node dst tile
  has near-uniform in-degree; each dst node owns one SBUF partition row and
  its (padded) incident edges sit along the free dim.  The segment softmax
  becomes plain free-dim reductions -- no masks, no per-edge matmuls.
- Edge payloads are fetched with batched dma_gather (int16 indices, <=1024
  per call, spread over 4 SWDGE queues).  The 50176-row node table exceeds
  the int16 range, so edges gather from two overlapping 32768-row windows
  ([0, 32768) and [17408, 50176)); edges with sources in the overlap are
  assigned to whichever window balances the per-row split.  Both windows'
  chunks land side by side in one SBUF tile so per-edge math is single-pass.
- Node tables are built per-core and AllGathered in 7 chunks interleaved
  with compute; the global table layout is chunk-major ([chunk][core][row])
  so each chunk's collective output is contiguous.
- Padded edge slots point at designated pad rows (one per window) whose
  a_src logit is -1000, so exp(leaky_relu(...)) underflows to exactly 0.
- Layer-2 outputs are scattered back to original node order with
  dma_scatter_add, then max-pooled per graph with precomputed segment masks
  and AllReduced (max) across cores.

Self-contained: hardcodes all shapes; reads nothing from /root/problem.
"""
import sys

sys.path.insert(0, "/opt/trn_rl_repo")
sys.path.insert(0, "/opt/pypackages")

import numpy as np

# ---------------------------------------------------------------- constants
N = 50000
IN_CH = 128
HID = 32
OUT_CH = 64
HEADS = 4
NCLS = 32
NG = 64
NC = 8
NEG = 0.2
P = 128
NLOC = N // NC              # 6250
TILES = -(-NLOC // P)       # 49
TPAD = TILES * P            # 6272
VPAD = NC * TPAD            # 50176
WIN = 32768
OFFB = VPAD - WIN           # 17408
CL1 = HEADS * HID           # 128
CL2 = HEADS * OUT_CH        # 256
R1 = 256                    # table1 row (halfs): h1(128) | as1(4) | pad
R2 = 384                    # table2 row (halfs): h2(256) | as2(4) | pad
POOLP = OUT_CH
OFF = 1024.0
NCH = 7                     # table chunks (collective granularity)
TPG = TILES // NCH          # 7 tiles per chunk
CHR = TPG * P               # 896 local rows per chunk
GCH = NC * CHR              # 7168 global rows per chunk
NPADF = 11                  # pad slots at the front of each core's order
MAXW = 8                    # dma_gather caps out near 1024 indices/call
NQ = 4                      # SWDGE queues

# chunk-major global row of (core k, local sorted row r)
def _gmap(k, r):
    return (r // CHR) * GCH + k * CHR + (r % CHR)


PAD_A = _gmap(0, 0)                 # = 0, in window A (front pad slot)
PAD_B = _gmap(0, TPAD - 1)          # = 43903, in window B (back pad slot)
assert PAD_A < WIN and PAD_B >= OFFB


# ---------------------------------------------------------------- host prep
def _sort_and_split(deg, src, dst):
    """Per-core degree sort (pads split front/back) + window assignment.

    Returns sp_cm[node] = core*TPAD + slot (core-major), g[node] = chunk-major
    global table row, per-node nA, and shared per-tile (WA, WB)."""
    def make_sorted_pos(keys):
        sp = np.empty(N, np.int64)
        for k in range(NC):
            lo = k * NLOC
            order = np.lexsort(tuple(kk[lo:lo + NLOC] for kk in reversed(keys)))
            sp[lo + order] = k * TPAD + NPADF + np.arange(NLOC)
        return sp

    def to_g(sp):
        k = sp // TPAD
        r = sp % TPAD
        return (r // CHR) * GCH + k * CHR + (r % CHR)

    def forced_counts(g_):
        srow = g_[src]
        nAf = np.bincount(dst, weights=(srow < OFFB).astype(np.float64),
                          minlength=N).astype(np.int64)
        nBf = np.bincount(dst, weights=(srow >= WIN).astype(np.float64),
                          minlength=N).astype(np.int64)
        return nAf, nBf

    sp = make_sorted_pos([-deg])
    nAf, nBf = forced_counts(to_g(sp))
    f = deg - nAf - nBf
    nA_prelim = nAf + np.clip((deg + 1) // 2 - nAf, 0, f)
    sp = make_sorted_pos([-deg, nA_prelim])
    g = to_g(sp)
    nAf, nBf = forced_counts(g)
    f = deg - nAf - nBf

    nAf_s = np.zeros(VPAD, np.int64)
    f_s = np.zeros(VPAD, np.int64)
    d_s = np.zeros(VPAD, np.int64)
    nAf_s[sp] = nAf
    f_s[sp] = f
    d_s[sp] = deg
    nAf_t = nAf_s.reshape(NC, TILES, P)
    f_t = f_s.reshape(NC, TILES, P)
    d_t = d_s.reshape(NC, TILES, P)
    WA = np.zeros(TILES, np.int64)
    WB = np.zeros(TILES, np.int64)
    nA_slot = np.zeros(VPAD, np.int64).reshape(NC, TILES, P)
    for t in range(TILES):
        best = None
        for tau in range(0, int(d_t[:, t].max()) + 2):
            nA = np.clip(tau, nAf_t[:, t], nAf_t[:, t] + f_t[:, t])
            wa = int(nA.max())
            wb = int((d_t[:, t] - nA).max())
            if best is None or wa + wb < best[0]:
                best = (wa + wb, wa, wb, tau)
        WA[t], WB[t] = best[1], best[2]
        nA_slot[:, t] = np.clip(best[3], nAf_t[:, t], nAf_t[:, t] + f_t[:, t])
    WA = np.maximum(WA, 1)
    WB = np.maximum(WB, 1)
    return sp, g, nA_slot.reshape(-1), [int(w) for w in WA], [int(w) for w in WB]


def _wrap16(arr, ni):
    """Index vector [ni] -> dma_gather idx layout [128, ni//16] int16."""
    a = arr.reshape(ni // 16, 16).T.astype(np.int16)
    return np.ascontiguousarray(np.tile(a, (8, 1)))


def _chunks(w):
    """Split a width into near-equal chunks of at most MAXW columns."""
    n = -(-w // MAXW)
    base = w // n
    return [base + (1 if i < w % n else 0) for i in range(n)]


def _build_eidx(sp, g, nA_slot, WAs, WBs, src, dst):
    """Per-core packed gather indices for both windows, all tiles, chunked."""
    srow = g[src]
    drow = sp[dst]
    eorder = np.argsort(drow, kind="stable")
    srow_s = srow[eorder]
    drow_s = drow[eorder]
    bounds = np.searchsorted(drow_s, np.arange(VPAD + 1))

    eidxA = [[] for _ in range(NC)]
    eidxB = [[] for _ in range(NC)]
    for k in range(NC):
        for t in range(TILES):
            WA, WB = WAs[t], WBs[t]
            arrA = np.full((WA, P), PAD_A, np.int64)
            arrB = np.full((WB, P), PAD_B - OFFB, np.int64)
            for p in range(P):
                gslot = k * TPAD + t * P + p
                a, b = bounds[gslot], bounds[gslot + 1]
                s = srow_s[a:b]
                inA = s < OFFB
                inB = s >= WIN
                flex = ~(inA | inB)
                nA = int(nA_slot[gslot])
                aa = nA - int(inA.sum())
                fi = np.flatnonzero(flex)
                listA = np.concatenate([s[inA], s[fi[:aa]]])
                listB = np.concatenate([s[inB], s[fi[aa:]]])
                arrA[:len(listA), p] = listA
                arrB[:len(listB), p] = listB - OFFB
            c0 = 0
            for w in _chunks(WA):
                eidxA[k].append(_wrap16(arrA[c0:c0 + w].reshape(-1), P * w))
                c0 += w
            c0 = 0
            for w in _chunks(WB):
                eidxB[k].append(_wrap16(arrB[c0:c0 + w].reshape(-1), P * w))
                c0 += w
    eidxA = [np.concatenate(e, axis=1) for e in eidxA]
    eidxB = [np.concatenate(e, axis=1) for e in eidxB]
    return eidxA, eidxB


def _build_pool(batch):
    """Per-core segment masks / graph-slot matrix on ORIGINAL node order."""
    runs = [[[] for _ in range(TILES)] for _ in range(NC)]
    SEG = 1
    for k in range(NC):
        bl = batch[k * NLOC:(k + 1) * NLOC].astype(np.int64)
        for t in range(TILES):
            lo = t * P
            hi = min((t + 1) * P, NLOC)
            if hi <= lo:
                continue
            seg = bl[lo:hi]
            cuts = np.flatnonzero(np.diff(seg)) + 1
            edges = np.concatenate([[0], cuts, [len(seg)]])
            for i in range(len(edges) - 1):
                runs[k][t].append((int(seg[edges[i]]), int(edges[i]), int(edges[i + 1])))
            SEG = max(SEG, len(edges) - 1)
    STOT = TILES * SEG
    segmask = np.zeros((NC, TILES, POOLP, SEG * P), np.float16)
    M = np.zeros((NC, POOLP, NG * STOT), np.float16)
    for k in range(NC):
        for t in range(TILES):
            for s, (gid, lo, hi) in enumerate(runs[k][t]):
                segmask[k, t, :, s * P + lo:s * P + hi] = 1.0
                M[k, :, gid * STOT + t * SEG + s] = 1.0
    return SEG, STOT, segmask, M


def _preprocess(x, edge_index, batch):
    src = np.concatenate([edge_index[0].astype(np.int64), np.arange(N, dtype=np.int64)])
    dst = np.concatenate([edge_index[1].astype(np.int64), np.arange(N, dtype=np.int64)])
    deg = np.bincount(dst, minlength=N)
    sp, g, nA_slot, WAs, WBs = _sort_and_split(deg, src, dst)
    eidxA, eidxB = _build_eidx(sp, g, nA_slot, WAs, WBs, src, dst)
    SEG, STOT, segmask, M = _build_pool(batch)

    # per-core sorted x, transposed fp16
    xT16 = []
    for k in range(NC):
        xs = np.zeros((TPAD, IN_CH), np.float32)
        loc = sp[k * NLOC:(k + 1) * NLOC] - k * TPAD
        xs[loc] = x[k * NLOC:(k + 1) * NLOC]
        xT16.append(np.ascontiguousarray(xs.T.astype(np.float16)))

    # scatter indices: sorted slot (p, tile c) -> original local row,
    # grouped into NCH scatter groups of TPG tiles
    so2 = []
    for k in range(NC):
        orig_of_slot = np.arange(TPAD, dtype=np.int64)  # dump rows for pads
        loc = sp[k * NLOC:(k + 1) * NLOC] - k * TPAD
        orig_of_slot[loc] = np.arange(NLOC)
        per_tile = orig_of_slot.reshape(TILES, P)
        grp = []
        for gg in range(NCH):
            seg = per_tile[gg * TPG:(gg + 1) * TPG]
            grp.append(_wrap16(seg.reshape(-1), TPG * P))
        so2.append(np.concatenate(grp, axis=1))

    meta = dict(WAs=WAs, WBs=WBs, SEG=SEG, STOT=STOT,
                CWA=8 * sum(WAs), CWB=8 * sum(WBs))
    per_core = [dict(eidxA=eidxA[k], eidxB=eidxB[k], so2=so2[k], xT16=xT16[k],
                     segmask=segmask[k].reshape(TILES * POOLP, SEG * P),
                     Mfull=M[k]) for k in range(NC)]
    return meta, per_core, sp


def _weights_host(sp, x, W1, a_s1, a_d1, b1, W2, a_s2, a_d2, b2, Wl, bl):
    f16, f32 = np.float16, np.float32
    B1as = np.zeros((IN_CH, HEADS), f32)
    B1ad = np.zeros((IN_CH, HEADS), f32)
    for h in range(HEADS):
        B1as[:, h] = W1[h * HID:(h + 1) * HID, :].T @ a_s1[h]
        B1ad[:, h] = W1[h * HID:(h + 1) * HID, :].T @ a_d1[h]
    B2as = np.zeros((CL1, HEADS), f32)
    B2ad = np.zeros((CL1, HEADS), f32)
    for h in range(HEADS):
        B2as[:, h] = W2[h * OUT_CH:(h + 1) * OUT_CH, :].T @ a_s2[h]
        B2ad[:, h] = W2[h * OUT_CH:(h + 1) * OUT_CH, :].T @ a_d2[h]

    WB1 = np.concatenate([W1.T, B1as], axis=1).astype(f16)          # [128, 132]
    WB2 = np.concatenate([W2.T, B2as, B2ad], axis=1).astype(f16)    # [128, 264]

    # per-core a_dst1 logits of local sorted nodes (computed on host)
    alS1 = []
    for k in range(NC):
        xs = np.zeros((TPAD, IN_CH), np.float32)
        loc = sp[k * NLOC:(k + 1) * NLOC] - k * TPAD
        xs[loc] = x[k * NLOC:(k + 1) * NLOC]
        ad1 = (xs @ B1ad).astype(f16).reshape(TILES, P, HEADS)
        alS1.append(np.ascontiguousarray(
            ad1.transpose(1, 0, 2).reshape(P, TILES * HEADS)))
    alS1 = [np.concatenate([a.astype(f32), a.astype(f32) * NEG], axis=1)
            for a in alS1]

    shared = dict(
        WB1=WB1, WB2=WB2,
        b1row=np.tile(b1.astype(f32)[None, :], (P, 1)),
        b2row=np.tile(b2.astype(f32)[None, :], (P, 1)),
        blrow=np.tile(bl.astype(f32)[None, :], (NG, 1)),
        WlT32=Wl.T.astype(f32).copy(),
        m1000=np.full((1, 4), -1000.0, f16),
    )
    return shared, alS1


# ---------------------------------------------------------------- program
def _build(meta):
    import concourse.bass as bass
    import concourse.bacc as bacc
    import concourse.tile as tile
    from concourse import mybir

    f16, f32, i16 = mybir.dt.float16, mybir.dt.float32, mybir.dt.int16
    Alu = mybir.AluOpType
    Act = mybir.ActivationFunctionType
    from concourse.masks import make_identity

    WAs, WBs = meta["WAs"], meta["WBs"]
    SEG, STOT = meta["SEG"], meta["STOT"]
    CWA, CWB = meta["CWA"], meta["CWB"]

    nc = bacc.Bacc("TRN2", target_bir_lowering=False, debug=False,
                   num_devices=NC, num_swdge_queues=NQ)

    pr = {}
    def param(name, shape, dt, out=False):
        pr[name] = nc.declare_dram_parameter(name, list(shape), dt, isOutput=out)
        return pr[name]

    param("xT16", [P, TPAD], f16)
    param("eidxA", [P, CWA], i16)
    param("eidxB", [P, CWB], i16)
    param("so2", [P, NCH * (TPG * P // 16)], i16)
    param("alS1", [P, 2 * TILES * HEADS], f32)
    param("segmask", [TILES * POOLP, SEG * P], f16)
    param("Mfull", [POOLP, NG * STOT], f16)
    param("WB1", [P, 132], f16)
    param("WB2", [P, 264], f16)
    param("b1row", [P, CL1], f32)
    param("b2row", [P, OUT_CH], f32)
    param("blrow", [NG, NCLS], f32)
    param("WlT32", [OUT_CH, NCLS], f32)
    param("m1000", [1, 4], f16)
    out_ext = param("out_logits", [NG, NCLS], f32, out=True)

    rg = [list(range(NC))]
    qctr = [0]
    def next_q():
        q = qctr[0] % NQ
        qctr[0] += 1
        return q

    with tile.TileContext(nc) as tc:
        with tc.tile_pool(name="const", bufs=1) as cp, \
             tc.tile_pool(name="dram", bufs=1, space="DRAM") as dp:

            def ld(name, shape, dt):
                t_ = cp.tile(list(shape), dt, name="c_" + name)
                nc.sync.dma_start(t_[:], pr[name][:])
                return t_

            WB1s = ld("WB1", [P, 132], f16)
            WB2s = ld("WB2", [P, 264], f16)
            b1row = ld("b1row", [P, CL1], f32)
            b2row = ld("b2row", [P, OUT_CH], f32)
            blrow = ld("blrow", [NG, NCLS], f32)
            WlT32 = ld("WlT32", [OUT_CH, NCLS], f32)
            m1000t = ld("m1000", [1, 4], f16)
            Mfull = ld("Mfull", [POOLP, NG * STOT], f16)
            idxA = ld("eidxA", [P, CWA], i16)
            idxB = ld("eidxB", [P, CWB], i16)
            so2t = ld("so2", [P, NCH * (TPG * P // 16)], i16)
            alS1 = ld("alS1", [P, 2 * TILES * HEADS], f32)

            ident = cp.tile([P, P], f32)
            make_identity(nc, ident[:])
            alS2 = cp.tile([P, 2 * TILES * HEADS], f32)
            o2acc = cp.tile([P, TILES * OUT_CH], f32)
            partial = cp.tile([POOLP, STOT], f32)

            tb1_loc = dp.tile([TPAD, R1], f16)
            tb1_full = dp.tile([VPAD, R1], f16)
            tb2_loc = dp.tile([TPAD, R2], f16)
            tb2_full = dp.tile([VPAD, R2], f16)
            o2_d = dp.tile([TPAD, OUT_CH], f32)
            ar_in = dp.tile([POOLP, NG], f32)
            ar_out = dp.tile([POOLP, NG], f32)

            # ============ phase T1: local table1 + chunked AllGather
            with tc.tile_pool(name="t1", bufs=4) as p1, \
                 tc.tile_pool(name="t1ps", bufs=4, space="PSUM") as p1ps:
                for t in range(TILES):
                    xsl = p1.tile([P, P], f16, tag="xsl")
                    nc.sync.dma_start(xsl[:], pr["xT16"][:, t * P:(t + 1) * P])
                    ps = p1ps.tile([P, 132], f32, tag="ps")
                    nc.tensor.matmul(ps[:], lhsT=xsl[:], rhs=WB1s[:],
                                     start=True, stop=True)
                    h16 = p1.tile([P, 132], f16, tag="h16")
                    nc.scalar.activation(h16[:], ps[:], Act.Copy)
                    nc.sync.dma_start(tb1_loc[t * P:(t + 1) * P, 0:132], h16[:])
                    if t == 0:
                        nc.sync.dma_start(tb1_loc[0:1, 128:132], m1000t[:])
                    if t == TILES - 1:
                        nc.sync.dma_start(tb1_loc[TPAD - 1:TPAD, 128:132],
                                          m1000t[:])
                    if t % TPG == TPG - 1:
                        c = t // TPG
                        nc.gpsimd.collective_compute(
                            "AllGather", mybir.AluOpType.bypass,
                            replica_groups=rg,
                            ins=[tb1_loc[c * CHR:(c + 1) * CHR, :].opt()],
                            outs=[tb1_full[c * GCH:(c + 1) * GCH, :].opt()])
            tc.strict_bb_all_engine_barrier()

            # ============ edge layer (shared for L1/L2)
            def edge_layer(t, pe, CL, ROW, tabw, colA, colB, adS, out_cb):
                WA, WB = WAs[t], WBs[t]
                W = WA + WB
                CH = CL // HEADS
                hg = pe.tile([P, W * ROW], f16, tag="hg")
                co = 0
                ca = colA
                for w in _chunks(WA):
                    nc.gpsimd.dma_gather(
                        out_ap=hg[:, co * ROW:(co + w) * ROW]
                            .rearrange("p (w r) -> p w r", w=w),
                        in_ap=tabw[0][:], idxs_ap=idxA[:, ca:ca + 8 * w],
                        num_idxs=P * w, num_idxs_reg=P * w, elem_size=ROW,
                        queue_num=next_q())
                    co += w
                    ca += 8 * w
                cb = colB
                for w in _chunks(WB):
                    nc.gpsimd.dma_gather(
                        out_ap=hg[:, co * ROW:(co + w) * ROW]
                            .rearrange("p (w r) -> p w r", w=w),
                        in_ap=tabw[1][:], idxs_ap=idxB[:, cb:cb + 8 * w],
                        num_idxs=P * w, num_idxs_reg=P * w, elem_size=ROW,
                        queue_num=next_q())
                    co += w
                    cb += 8 * w
                hg3 = hg[:].rearrange("p (w r) -> p w r", w=W)
                ad = adS[:, HEADS * t:HEADS * (t + 1)]
                ads = adS[:, TILES * HEADS + HEADS * t:
                          TILES * HEADS + HEADS * (t + 1)]
                # exp(leaky_relu(as+ad)) == max(exp(as+ad), exp(NEG*(as+ad)))
                e1 = pe.tile([P, HEADS * W], f16, tag="e1")
                e2 = pe.tile([P, HEADS * W], f16, tag="e2")
                for h in range(HEADS):
                    asg_h = hg3[:, :, CL + h:CL + h + 1]                         .rearrange("p w o -> p (w o)")
                    nc.scalar.activation(e1[:, h * W:(h + 1) * W], asg_h,
                                         Act.Exp, bias=ad[:, h:h + 1])
                    nc.scalar.activation(e2[:, h * W:(h + 1) * W], asg_h,
                                         Act.Exp, bias=ads[:, h:h + 1],
                                         scale=NEG)
                ex = pe.tile([P, HEADS * W], f16, tag="ex")
                nc.vector.tensor_tensor(out=ex[:], in0=e1[:], in1=e2[:],
                                        op=Alu.max)
                den = pe.tile([P, HEADS], f32, tag="den")
                nc.vector.reduce_sum(den[:],
                                     ex[:].rearrange("p (h w) -> p h w", h=HEADS),
                                     axis=mybir.AxisListType.X)
                dei = pe.tile([P, HEADS], f32, tag="dei")
                nc.vector.tensor_scalar_add(out=dei[:], in0=den[:], scalar1=1e-16)
                rden = pe.tile([P, HEADS], f32, tag="rden")
                nc.vector.reciprocal(rden[:], dei[:])
                w1 = pe.tile([P, CL * W], f16, tag="w1")
                nc.vector.tensor_tensor(
                    out=w1[:].rearrange("p (h c w) -> p h c w", h=HEADS, c=CH),
                    in0=hg3[:, :, 0:CL].rearrange("p w (h c) -> p h c w", h=HEADS),
                    in1=ex[:].rearrange("p (h w) -> p h w", h=HEADS)[:, :, None, :]
                        .to_broadcast([P, HEADS, CH, W]),
                    op=Alu.mult)
                nums = pe.tile([P, CL], f32, tag="nums")
                nc.vector.reduce_sum(nums[:],
                                     w1[:].rearrange("p (c w) -> p c w", c=CL),
                                     axis=mybir.AxisListType.X)
                num = pe.tile([P, CL], f32, tag="num")
                nc.vector.tensor_tensor(
                    out=num[:].rearrange("p (h c) -> p h c", h=HEADS),
                    in0=nums[:].rearrange("p (h c) -> p h c", h=HEADS),
                    in1=rden[:, :, None].to_broadcast([P, HEADS, CH]),
                    op=Alu.mult)
                out_cb(t, num, pe)

            # ============ layer 1 (epilogue builds table2, chunked AllGather)
            with tc.tile_pool(name="l1", bufs=4) as pe1, \
                 tc.tile_pool(name="l1ps", bufs=2, space="PSUM") as pe1ps:
                colA = colB = 0
                def epi1(t, num, pe):
                    u32 = pe.tile([P, CL1], f32, tag="u32")
                    nc.vector.tensor_tensor(out=u32[:], in0=num[:], in1=b1row[:],
                                            op=Alu.add)
                    ur = pe.tile([P, CL1], f32, tag="ur")
                    nc.vector.tensor_scalar(out=ur[:], in0=u32[:], scalar1=0.0,
                                            scalar2=None, op0=Alu.max)
                    uTps = pe1ps.tile([P, P], f32, tag="uTps")
                    nc.tensor.transpose(out=uTps[:], in_=ur[:], identity=ident[:])
                    uT16 = pe.tile([P, P], f16, tag="uT16")
                    nc.scalar.activation(uT16[:], uTps[:], Act.Copy)
                    ps2 = pe1ps.tile([P, 264], f32, tag="ps2")
                    nc.tensor.matmul(ps2[:], lhsT=uT16[:], rhs=WB2s[:],
                                     start=True, stop=True)
                    h216 = pe.tile([P, 264], f16, tag="h216")
                    nc.scalar.activation(h216[:], ps2[:], Act.Copy)
                    nc.sync.dma_start(tb2_loc[t * P:(t + 1) * P, 0:260],
                                      h216[:, 0:260])
                    nc.vector.tensor_copy(out=alS2[:, HEADS * t:HEADS * (t + 1)],
                                          in_=h216[:, 260:264])
                    nc.vector.tensor_scalar(
                        out=alS2[:, TILES * HEADS + HEADS * t:
                                 TILES * HEADS + HEADS * (t + 1)],
                        in0=h216[:, 260:264], scalar1=NEG, scalar2=None,
                        op0=Alu.mult)
                    if t == 0:
                        nc.sync.dma_start(tb2_loc[0:1, 256:260], m1000t[:])
                    if t == TILES - 1:
                        nc.sync.dma_start(tb2_loc[TPAD - 1:TPAD, 256:260],
                                          m1000t[:])

                for t in range(TILES):
                    edge_layer(t, pe1, CL1, R1,
                               (tb1_full[0:WIN, :], tb1_full[OFFB:VPAD, :]),
                               colA, colB, alS1, epi1)
                    colA += 8 * WAs[t]
                    colB += 8 * WBs[t]

            for c in range(NCH):
                nc.gpsimd.collective_compute(
                    "AllGather", mybir.AluOpType.bypass, replica_groups=rg,
                    ins=[tb2_loc[c * CHR:(c + 1) * CHR, :].opt()],
                    outs=[tb2_full[c * GCH:(c + 1) * GCH, :].opt()])
            # chunked-slice collectives are not reliably dependency-tracked
            # against the custom-DMA gathers that read their outputs; pin
            # ordering explicitly
            tc.strict_bb_all_engine_barrier()

            # ============ layer 2 (big-W head tiles at bufs=2, tail at bufs=3)
            def epi2(t, num, pe):
                o2p = pe.tile([P, OUT_CH], f32, tag="o2p")
                nc.vector.reduce_sum(
                    o2p[:],
                    num[:].rearrange("p (h c) -> p c h", h=HEADS),
                    axis=mybir.AxisListType.X)
                nc.vector.scalar_tensor_tensor(
                    out=o2acc[:, OUT_CH * t:OUT_CH * (t + 1)],
                    in0=o2p[:], scalar=1.0 / HEADS, in1=b2row[:],
                    op0=Alu.mult, op1=Alu.add)

            TSPLIT = next((t for t in range(TILES) if WAs[t] + WBs[t] <= 28),
                          TILES)
            colA = colB = 0
            with tc.tile_pool(name="l2a", bufs=2) as pe2a:
                for t in range(TSPLIT):
                    edge_layer(t, pe2a, CL2, R2,
                               (tb2_full[0:WIN, :], tb2_full[OFFB:VPAD, :]),
                               colA, colB, alS2, epi2)
                    colA += 8 * WAs[t]
                    colB += 8 * WBs[t]
            with tc.tile_pool(name="l2b", bufs=3) as pe2b:
                for t in range(TSPLIT, TILES):
                    edge_layer(t, pe2b, CL2, R2,
                               (tb2_full[0:WIN, :], tb2_full[OFFB:VPAD, :]),
                               colA, colB, alS2, epi2)
                    colA += 8 * WAs[t]
                    colB += 8 * WBs[t]

            # ============ scatter to original order + pooled max + head
            with tc.tile_pool(name="fin", bufs=2) as fp, \
                 tc.tile_pool(name="finps", bufs=2, space="PSUM") as fps:
                zt = fp.tile([P, TILES * OUT_CH], f32, tag="zt")
                nc.vector.memset(zt[:], 0.0)
                nc.sync.dma_start(
                    o2_d[:].rearrange("(a b) r -> a (b r)", b=TILES), zt[:])
                tc.strict_bb_all_engine_barrier()
                GW = TPG * P // 16
                for gg in range(NCH):
                    nc.gpsimd.dma_scatter_add(
                        out_ap=o2_d[:],
                        in_ap=o2acc[:, gg * TPG * OUT_CH:(gg + 1) * TPG * OUT_CH]
                            .rearrange("p (t r) -> p t r", t=TPG),
                        idxs_ap=so2t[:, gg * GW:(gg + 1) * GW],
                        num_idxs=TPG * P, num_idxs_reg=TPG * P,
                        elem_size=OUT_CH, queue_num=next_q())
                tc.strict_bb_all_engine_barrier()

                for t in range(TILES):
                    o2l = fp.tile([P, OUT_CH], f32, tag="o2l")
                    nc.sync.dma_start(o2l[:], o2_d[t * P:(t + 1) * P, :])
                    o2Tps = fps.tile([POOLP, P], f32, tag="o2Tps")
                    nc.tensor.transpose(out=o2Tps[:], in_=o2l[:], identity=ident[:])
                    o2T = fp.tile([POOLP, P], f32, tag="o2T")
                    nc.scalar.activation(o2T[:], o2Tps[:], Act.Copy)
                    sm = fp.tile([POOLP, SEG * P], f16, tag="sm")
                    nc.sync.dma_start(sm[:], pr["segmask"][t * POOLP:(t + 1) * POOLP, :])
                    tmp = fp.tile([POOLP, SEG * P], f32, tag="ptmp")
                    nc.vector.scalar_tensor_tensor(
                        out=tmp[:].rearrange("q (s d) -> q s d", s=SEG),
                        in0=o2T[:][:, None, :].to_broadcast([POOLP, SEG, P]),
                        scalar=OFF, in1=sm[:].rearrange("q (s d) -> q s d", s=SEG),
                        op0=Alu.add, op1=Alu.mult)
                    nc.vector.reduce_max(
                        partial[:, t * SEG:(t + 1) * SEG],
                        tmp[:].rearrange("q (s d) -> q s d", s=SEG),
                        axis=mybir.AxisListType.X)

                tmpg = fp.tile([POOLP, NG * STOT], f32, tag="tmpg")
                nc.vector.scalar_tensor_tensor(
                    out=tmpg[:], in0=partial[:][:, None, :].to_broadcast([POOLP, NG, STOT]),
                    scalar=0.0, in1=Mfull[:].rearrange("q (g s) -> q g s", g=NG),
                    op0=Alu.add, op1=Alu.mult)
                pooled = fp.tile([POOLP, NG], f32, tag="pooled")
                nc.vector.reduce_max(pooled[:],
                                     tmpg[:].rearrange("q (g s) -> q g s", g=NG),
                                     axis=mybir.AxisListType.X)
                pooled2 = fp.tile([POOLP, NG], f32, tag="pooled2")
                nc.vector.tensor_scalar_add(out=pooled2[:], in0=pooled[:],
                                            scalar1=-OFF)
                nc.sync.dma_start(ar_in[:], pooled2[:])
                nc.gpsimd.collective_compute(
                    "AllReduce", mybir.AluOpType.max, replica_groups=rg,
                    ins=[ar_in[:].opt()], outs=[ar_out[:].opt()])
                pooledF = fp.tile([POOLP, NG], f32, tag="pooledF")
                nc.sync.dma_start(pooledF[:], ar_out[:])
                lps = fps.tile([NG, NCLS], f32, tag="lps")
                nc.tensor.matmul(lps[:], lhsT=pooledF[:], rhs=WlT32[:],
                                 start=True, stop=True)
                logits = fp.tile([NG, NCLS], f32, tag="logits")
                nc.vector.tensor_tensor(out=logits[:], in0=lps[:], in1=blrow[:],
                                        op=Alu.add)
                nc.sync.dma_start(out_ext[:], logits[:])

    nc.compile()
    return nc


# ---------------------------------------------------------------- runner
def _kernel_impl(inputs, trace=False, trace_kwargs=None):
    x = np.asarray(inputs["x"], np.float32)
    edge_index = np.asarray(inputs["edge_index"]).astype(np.int64)
    batch = np.asarray(inputs["batch"]).astype(np.int64)
    meta, per_core, sp = _preprocess(x, edge_index, batch)
    shared, alS1 = _weights_host(
        sp, x,
        np.asarray(inputs["W1"], np.float32), np.asarray(inputs["a_src1"], np.float32),
        np.asarray(inputs["a_dst1"], np.float32), np.asarray(inputs["b1"], np.float32),
        np.asarray(inputs["W2"], np.float32), np.asarray(inputs["a_src2"], np.float32),
        np.asarray(inputs["a_dst2"], np.float32), np.asarray(inputs["b2"], np.float32),
        np.asarray(inputs["Wl"], np.float32), np.asarray(inputs["bl"], np.float32))
    nc = _build(meta)
    in_maps = []
    for k in range(NC):
        m = dict(shared)
        m.update(per_core[k])
        m["alS1"] = alS1[k]
        in_maps.append({kk: np.ascontiguousarray(v) for kk, v in m.items()})
    from concourse.bass_utils import run_bass_kernel_spmd
    res = run_bass_kernel_spmd(nc, in_maps, core_ids=list(range(NC)),
                               trace=trace, **(trace_kwargs or {}))
    return np.asarray(res.results[0]["out_logits"], np.float32), res


def kernel(**inputs):
    return _kernel_impl(inputs)[0]
